# revision 27
# baseline (speedup 1.0000x reference)
"""Multi-head attention (softmax over the query axis) on 8 trn2 cores.

Sharding: tensor-parallel over heads — 2 heads per core. Each core computes
its heads' projections + attention + a partial output projection (row-parallel
Wo); the host sums the 8 partial outputs and adds bo.

Device-side layout choices (host pre-packs everything):
  - activations are shipped TRANSPOSED (d on partitions) as bf16, so every
    matmul contracts over the partition dim with natural-layout DMAs.
  - scores are computed transposed ([t, s]) so the softmax axis (query s) is
    the free axis: one exp-activation per strip with fused row-sum.
  - 1/rowsum is folded into V's rows (16K elems) instead of the attention
    matrix (4.2M elems).

Perf notes (measured on HW via NTFF profiles):
  - dependency-free warm-up matmuls bridge the initial DMA ramp so the PE's
    HAM clock gate reaches 2.4 GHz before real work starts (idle >3.4us
    re-throttles the array to 1.2 GHz).
  - consecutive duplicate Ldweights are elided at BIR level (bass emits one
    per matmul; the array retains the stationary operand).
  - output leaves in [128, 2048] bf16 tiles (4KB DMA rows) split across both
    HWDGE rings; the final tile drains half-per-ring.
  - O-phase loops h-outer so each head's HT strip loads once per st; a few
    pass2-h0 strips run early (inside PV+S0) to keep the ACT exp stream
    ahead of pass2-h1's weight loads.
"""

import json

import numpy as np
import ml_dtypes

import concourse.bass as bass
import concourse.mybir as mybir
import concourse.tile as tile
from concourse import bass_utils

BF16 = mybir.dt.bfloat16
F32 = mybir.dt.float32
AF = mybir.ActivationFunctionType

N_CORES = 8
H = 16
D = 2048
DK = 128
S = 2048
HPC = H // N_CORES          # heads per core = 2
NT = D // 128               # 16 tiles along d / t
NSC = S // 512              # 4 chunks of 512 along s / m
SCALE = 1.0 / float(np.sqrt(DK))

TRACE = False
LAST_RESULTS = None
PHASE_MARKS = []


def _mark(nc, label):
    PHASE_MARKS.append((label, nc.next_id()))


# The walrus in this container accepts only ONE sem-wait per instruction
# (setupSyncWait: "Too many sync wait commands"), but Tile attaches one wait
# per depended-on semaphore. Split extra waits onto single-wait NoOps inserted
# just before the instruction on the same engine, at BIR-JSON level so every
# compile path (native + bass2jax/axon) is covered.
def _split_multi_waits(raw: bytes) -> bytes:
    m = json.loads(raw)
    ctr = 0
    changed = False
    for fn in m.get("functions", []):
        for blk in fn.get("blocks", []):
            insts = blk.get("instructions", [])
            out = []
            for inst in insts:
                si = inst.get("sync_info")
                waits = (si.get("on_wait") or []) if si else []
                if len(waits) > 1:
                    changed = True
                    for w in waits[:-1]:
                        ctr += 1
                        out.append(
                            {
                                "debug": inst.get("debug"),
                                "engine": inst["engine"],
                                "ins": [],
                                "name": f"I-wsplit-{ctr}",
                                "opcode": "NoOp",
                                "outs": [],
                                "sync_info": {"on_update": [], "on_wait": [w]},
                            }
                        )
                    si["on_wait"] = [waits[-1]]
                out.append(inst)
            if changed:
                blk["instructions"] = out
    if not changed:
        return raw
    return json.dumps(m).encode()


# PE weight loads serialize with matmuls (~53ns each with FWL), and bass
# emits one Ldweights per Matmult even when consecutive matmuls share the
# same stationary operand (the array retains weights between matmuls — this
# is exactly what walrus's disabled --enable-ldw-opt pass would do, but that
# pass rejects pre-split Ldweights). Elide consecutive duplicates at BIR
# level, merging any waits into the paired Matmult (all updates already live
# on the Matmults; verified: every Ldweights is immediately followed by its
# Matmult in the scheduled stream).
def _elide_dup_ldweights(m: dict) -> None:
    for fn in m.get("functions", []):
        for blk in fn.get("blocks", []):
            insts = blk.get("instructions", [])
            out = []
            loaded = None  # weights key currently in the PE array
            pend_waits = None  # waits from an elided Ldweights
            for inst in insts:
                if inst.get("engine") != "PE":
                    out.append(inst)
                    continue
                op = inst.get("opcode")
                if op == "Ldweights":
                    key = json.dumps(
                        [
                            inst.get("ins"),
                            inst.get("tile_position"),
                            inst.get("tile_size"),
                            inst.get("perf_mode"),
                            inst.get("is_transpose"),
                        ],
                        sort_keys=True,
                    )
                    si = inst.get("sync_info") or {}
                    if key == loaded and not (si.get("on_update") or []):
                        w = si.get("on_wait") or []
                        if w:
                            pend_waits = (pend_waits or []) + w
                        continue  # drop the duplicate load
                    loaded = key
                    out.append(inst)
                elif op == "Matmult":
                    if pend_waits:
                        si = inst.setdefault("sync_info", {})
                        si["on_wait"] = (si.get("on_wait") or []) + pend_waits
                        pend_waits = None
                    out.append(inst)
                else:
                    # branches/drains/etc: conservatively forget array state
                    loaded = None
                    out.append(inst)
            blk["instructions"] = out


_orig_to_json_bytes = bass.Bass.to_json_bytes


def _to_json_bytes_split(self):
    m = json.loads(_orig_to_json_bytes(self))
    _elide_dup_ldweights(m)
    return _split_multi_waits(json.dumps(m).encode())


bass.Bass.to_json_bytes = _to_json_bytes_split


def _build_bass(loop_n=None):
    nc = bass.Bass(trn_type="TRN2")

    qT = nc.dram_tensor("qT", [D, S], BF16, kind="ExternalInput")
    kT = nc.dram_tensor("kT", [D, S], BF16, kind="ExternalInput")
    vT = nc.dram_tensor("vT", [D, S], BF16, kind="ExternalInput")
    wq = nc.dram_tensor("wq", [128, HPC * NT * 128], BF16, kind="ExternalInput")
    wk = nc.dram_tensor("wk", [128, HPC * NT * 128], BF16, kind="ExternalInput")
    wv2 = nc.dram_tensor("wv2", [128, NT * HPC * 128], BF16, kind="ExternalInput")
    wo = nc.dram_tensor("wo", [128, HPC * D], BF16, kind="ExternalInput")
    bqk = nc.dram_tensor("bqk", [128, 2 * HPC], F32, kind="ExternalInput")
    bvb = nc.dram_tensor("bvb", [128, HPC * 128], F32, kind="ExternalInput")
    out = nc.dram_tensor("out_p", [S, D], BF16, kind="ExternalOutput")

    with tile.TileContext(nc) as tc:
        with (
            tc.tile_pool(name="wpool", bufs=1) as wpool,
            tc.tile_pool(name="acts", bufs=1) as acts,
            tc.tile_pool(name="xpool", bufs=4) as xpool,
            tc.tile_pool(name="small", bufs=2) as small,
            tc.tile_pool(name="opool", bufs=2) as opool,
            tc.tile_pool(name="exppool", bufs=1) as exppool,
        ):
            # --- resident weights ---
            wq_sb = wpool.tile([128, HPC * NT * 128], BF16)
            wk_sb = wpool.tile([128, HPC * NT * 128], BF16)
            wv2_sb = wpool.tile([128, NT * HPC * 128], BF16)
            wo_sb = wpool.tile([128, HPC * D], BF16)
            bqk_sb = wpool.tile([128, 2 * HPC], F32)
            bvb_sb = wpool.tile([128, HPC * 128], F32)

            # --- resident per-head activations ---
            QT = [acts.tile([128, S], BF16, name=f"QT{h}") for h in range(HPC)]
            KT = [acts.tile([128, S], BF16, name=f"KT{h}") for h in range(HPC)]
            V = [acts.tile([128, NT * 128], BF16, name=f"V{h}") for h in range(HPC)]
            HT = [acts.tile([128, S], BF16, name=f"HT{h}") for h in range(HPC)]

            # Activation strips stream full-width (4KB rows -> fewest DMA
            # descriptors) on the SP DGE queue; ALL weight loads go through
            # the Activation DGE queue so their triggers issue in parallel
            # with (never behind) the strip stream. The 16 hardware DMA
            # queues are shared, but descriptor generation is per-engine.
            # wq/wk are packed strip-major (col = (dt*HPC+h)*128); wq is
            # DMA'd in three chunks so the first matmuls only wait on the
            # dt=0 strips, and the first activation strip lands in halves.
            xs_first = xpool.tile([128, S], BF16, name="xs", tag="xs", bufs=8)
            nc.sync.dma_start(xs_first[:, 0:1024], qT[0:128, 0:1024])
            nc.sync.dma_start(xs_first[:, 1024:2048], qT[0:128, 1024:2048])
            WQQ = HPC * NT * 32
            nc.scalar.dma_start(wq_sb[:, 0:256], wq[:, 0:256])
            nc.scalar.dma_start(wq_sb[:, 256:WQQ], wq[:, 256:WQQ])
            nc.scalar.dma_start(bqk_sb[:], bqk[:])

            # The PE's HAM clock gate keeps the array at 1.2 GHz until it has
            # been busy ~3.4us, and the initial DMA wait (~7us) resets that
            # window -- the whole first phase then runs at half clock. Bridge
            # the ramp with dependency-free warm-up matmuls on scratch data so
            # the PE is warm (2.4 GHz) the moment real operands land.
            with (
                tc.tile_pool(name="wusb", bufs=1) as wusb,
                tc.tile_pool(name="wups", bufs=1, space="PSUM") as wups,
            ):
                wsrc = wusb.tile([128, 256], BF16)
                nc.vector.memset(wsrc[:], 0.0)
                wp = wups.tile([128, 256], F32)
                NWU = 24
                for i in range(NWU):
                    nc.tensor.matmul(
                        wp[:], wsrc[:, 0:128], wsrc[:], start=(i == 0), stop=(i == NWU - 1)
                    )

            # benchmark mode: run the whole body loop_n times in one NEFF to
            # amortize dispatch overhead; weights loaded once up front.
            loop_ctx = None
            if loop_n:
                nc.sync.dma_start(wk_sb[:], wk[:])
                nc.sync.dma_start(wv2_sb[:], wv2[:])
                nc.sync.dma_start(bvb_sb[:], bvb[:])
                nc.sync.dma_start(wo_sb[:], wo[:])
                loop_ctx = tc.For_i(0, loop_n, 1)
                loop_ctx.__enter__()

            # ---------------- phase P-QK: Q^T / K^T projections ---------------
            # Full-strip loads ([128, S] = 4KB lines); one psum bank per
            # (head, s-chunk), accumulated across all 16 d-strips.
            with tc.tile_pool(name="ppqk", bufs=1, space="PSUM") as ppqk:
                for xdram, w_sb, dst, bcol in ((qT, wq_sb, QT, 0), (kT, wk_sb, KT, HPC)):
                    _mark(nc, "P-Q" if xdram is qT else "P-K")
                    if xdram is kT and not loop_n:
                        # on the SP queue: paced behind the q-strip triggers
                        # (their buffer-reuse waits), so it cannot crowd out
                        # the strip stream during the DMA ramp
                        nc.sync.dma_start(wk_sb[:], wk[:])
                    ps = [
                        [
                            ppqk.tile(
                                [128, 512], F32, name=f"pp{h}{c}", tag=f"pp{h}{c}", bufs=1
                            )
                            for c in range(NSC)
                        ]
                        for h in range(HPC)
                    ]
                    for dt in range(NT):
                        if xdram is qT and dt == 0:
                            xs = xs_first
                        else:
                            xs = xpool.tile([128, S], BF16, name="xs", tag="xs", bufs=8)
                            nc.sync.dma_start(xs[:], xdram[dt * 128 : (dt + 1) * 128, :])
                        if xdram is qT and not loop_n:
                            # wv2/bvb on the SP queue mid-loop: the strip
                            # triggers ahead of them carry buffer-reuse waits,
                            # so these fire mid-phase (after the DMA ramp)
                            # instead of crowding the ramp like the ungated
                            # Activation DGE queue would
                            if dt == 3:
                                # bulk of wq rides the strip ring BEHIND the
                                # first strips: the ramp delivers strips at
                                # full bandwidth, and this still lands well
                                # before the dt=4 matmuls need it
                                nc.sync.dma_start(
                                    wq_sb[:, WQQ : 4 * WQQ], wq[:, WQQ : 4 * WQQ]
                                )
                            if dt == 6:
                                nc.sync.dma_start(
                                    wv2_sb[:, 0 : NT * HPC * 64],
                                    wv2[:, 0 : NT * HPC * 64],
                                )
                            elif dt == 8:
                                nc.sync.dma_start(
                                    wv2_sb[:, NT * HPC * 64 : NT * HPC * 128],
                                    wv2[:, NT * HPC * 64 : NT * HPC * 128],
                                )
                            elif dt == 10:
                                nc.sync.dma_start(bvb_sb[:], bvb[:])
                        for h in range(HPC):
                            for c in range(NSC):
                                nc.tensor.matmul(
                                    ps[h][c][:],
                                    w_sb[:, (dt * HPC + h) * 128 : (dt * HPC + h + 1) * 128],
                                    xs[:, c * 512 : (c + 1) * 512],
                                    start=(dt == 0),
                                    stop=(dt == NT - 1),
                                )
                    # psum -> sbuf with bias, in bank order with alternating
                    # engines so every bank drains (and is reusable by the
                    # next phase) shortly after its own last matmul.
                    # (GpSimd/Pool cannot read PSUM — compile-rejected.)
                    for h in range(HPC):
                        for c in range(NSC):
                            if c % 2 == 0:
                                nc.scalar.activation(
                                    dst[h][:, c * 512 : (c + 1) * 512],
                                    ps[h][c][:],
                                    AF.Identity,
                                    bias=bqk_sb[:, bcol + h : bcol + h + 1],
                                    scale=1.0,
                                )
                            else:
                                nc.vector.tensor_scalar_add(
                                    dst[h][:, c * 512 : (c + 1) * 512],
                                    ps[h][c][:],
                                    bqk_sb[:, bcol + h : bcol + h + 1],
                                )

            # ------- phases P-V and S interleaved on one psum pool -----------
            # P-V shares the ph* psum tags (1 bank each) with S pass 2; the V
            # matmul groups are interleaved with S-h0 pass 1 so the PE stays
            # busy under pass 1's ACT-bound exp stream.
            with tc.tile_pool(name="pps", bufs=1, space="PSUM") as pps:
                if not loop_n:
                    nc.scalar.dma_start(wo_sb[:], wo[:])
                vsca = [
                    small.tile([128, NT * 128], BF16, name=f"vsca{h}", tag=f"vsca{h}", bufs=1)
                    for h in range(HPC)
                ]
                expts = {}

                def emit_v_group(tg):
                    # V: [t, hk] natural layout, both heads fused per matmul
                    psv = [
                        pps.tile(
                            [128, HPC * 128], F32, name=f"psv{tt}", tag=f"ph{tt}", bufs=1
                        )
                        for tt in range(4)
                    ]
                    for dt in range(NT):
                        xc = xpool.tile([128, 512], BF16, name="xc", tag="xc", bufs=12)
                        nc.sync.dma_start(
                            xc[:], vT[dt * 128 : (dt + 1) * 128, tg * 512 : (tg + 1) * 512]
                        )
                        for tt in range(4):
                            nc.tensor.matmul(
                                psv[tt][:],
                                xc[:, tt * 128 : (tt + 1) * 128],
                                wv2_sb[:, dt * HPC * 128 : (dt + 1) * HPC * 128],
                                start=(dt == 0),
                                stop=(dt == NT - 1),
                            )
                    for tt in range(4):
                        t_tile = tg * 4 + tt
                        for h in range(HPC):
                            nc.vector.tensor_tensor(
                                V[h][:, t_tile * 128 : (t_tile + 1) * 128],
                                psv[tt][:, h * 128 : (h + 1) * 128],
                                bvb_sb[:, h * 128 : (h + 1) * 128],
                                op=mybir.AluOpType.add,
                            )

                rects = {}

                def emit_pass1_partA(h, i):
                    # scores -> exp (+row-sum) -> 1/rowsum
                    expt = exppool.tile(
                        [128, S], BF16, name=f"expt{i}", tag=f"exp{i}", bufs=1
                    )
                    expts[(h, i)] = expt
                    sumt = small.tile([128, 2], F32, name="sumt", tag="sum", bufs=4)
                    for half in range(2):
                        psc = pps.tile(
                            [128, 1024], F32, name=f"psc{half}", tag=f"psc{half}", bufs=1
                        )
                        for cc in range(2):
                            c = half * 2 + cc
                            nc.tensor.matmul(
                                psc[:, cc * 512 : (cc + 1) * 512],
                                KT[h][:, i * 128 : (i + 1) * 128],
                                QT[h][:, c * 512 : (c + 1) * 512],
                                start=True,
                                stop=True,
                            )
                        nc.scalar.activation(
                            expt[:, half * 1024 : (half + 1) * 1024],
                            psc[:],
                            AF.Exp,
                            scale=SCALE,
                            accum_out=sumt[:, half : half + 1],
                        )
                    rect = small.tile(
                        [128, 1], F32, name="rect", tag=f"rec{i % 4}", bufs=2
                    )
                    rects[(h, i)] = rect
                    nc.vector.reduce_sum(rect[:], sumt[:], axis=mybir.AxisListType.X)
                    nc.vector.reciprocal(rect[:], rect[:])

                def emit_pass1_partB(h, i):
                    # fold 1/rowsum into this strip's V rows
                    nc.vector.tensor_scalar_mul(
                        vsca[h][:, i * 128 : (i + 1) * 128],
                        V[h][:, i * 128 : (i + 1) * 128],
                        rects[(h, i)][:],
                    )

                def emit_pass1_strip(h, i):
                    emit_pass1_partA(h, i)
                    emit_pass1_partB(h, i)

                def emit_pass2_strip(h, ph, i):
                    for c in range(NSC):
                        nc.tensor.matmul(
                            ph[c][:],
                            vsca[h][:, i * 128 : (i + 1) * 128],
                            expts[(h, i)][:, c * 512 : (c + 1) * 512],
                            start=(i == 0),
                            stop=(i == NT - 1),
                        )

                def emit_ht_copies(h, ph):
                    # h0's copies run while the Scalar engine is still
                    # saturated with h1's exp stream -> keep them off Scalar.
                    # h1's copies are on the O-phase critical path and the exp
                    # stream is done -> split across both engines.
                    for c in range(NSC):
                        if h == 1 and c % 2 == 1:
                            nc.scalar.copy(
                                HT[h][:, c * 512 : (c + 1) * 512], ph[c][:]
                            )
                        else:
                            nc.vector.tensor_copy(
                                HT[h][:, c * 512 : (c + 1) * 512], ph[c][:]
                            )

                def emit_pass2(h):
                    ph = [
                        pps.tile([128, 512], F32, name=f"ph{c}", tag=f"ph{c}", bufs=1)
                        for c in range(NSC)
                    ]
                    for i in range(NT):
                        emit_pass2_strip(h, ph, i)
                    emit_ht_copies(h, ph)

                _mark(nc, "PV+S0")
                # Interleave: strips' scores+exp first (feed ACT), V matmul
                # group fills PE during the exps, then the vsc muls that need
                # this group's V tiles.
                rects = {}
                for g in range(4):
                    for i in range(4 * g, 4 * g + 4):
                        emit_pass1_partA(0, i)
                    emit_v_group(g)
                    for i in range(4 * g, 4 * g + 4):
                        emit_pass1_partB(0, i)
                # a few pass2-h0 strips run at the tail of PV+S0 (PE-cheap
                # there) so the S2h0+S1h1 phase has less PE work than ACT
                # work -- otherwise the h1 exp stream finishes after pass2-h0
                # and S2-h1's first weight loads stall on it
                EARLY = 3
                ph0 = [
                    pps.tile([128, 512], F32, name=f"ph{c}", tag=f"ph{c}", bufs=1)
                    for c in range(NSC)
                ]
                for i in range(EARLY):
                    emit_pass2_strip(0, ph0, i)
                _mark(nc, "S2h0+S1h1")
                # pass2 of head 0 (PE-dense) strip-interleaved with pass1 of
                # head 1 (ACT-bound)
                for i in range(NT):
                    if i >= EARLY:
                        emit_pass2_strip(0, ph0, i)
                    emit_pass1_partA(1, i)
                    emit_pass1_partB(1, i)
                emit_ht_copies(0, ph0)
                _mark(nc, "S2-h1")
                emit_pass2(1)

            # ---------------- phase O: partial output projection --------------
            with tc.tile_pool(name="ppo", bufs=4, space="PSUM") as ppo:
                _mark(nc, "O")
                for st in range(NT):
                    # h-outer so each head's stationary HT strip is loaded
                    # once per st (the Ldweights-dedup BIR pass elides the
                    # repeats) instead of alternating HT0/HT1 every matmul.
                    pos = [
                        ppo.tile([128, 1024], F32, name="po", tag="po", bufs=4)
                        for _ in range(2)
                    ]
                    for h in range(HPC):
                        for cp in range(2):
                            for cc in range(2):
                                c = cp * 2 + cc
                                nc.tensor.matmul(
                                    pos[cp][:, cc * 512 : (cc + 1) * 512],
                                    HT[h][:, st * 128 : (st + 1) * 128],
                                    wo_sb[:, h * D + c * 512 : h * D + (c + 1) * 512],
                                    start=(h == 0),
                                    stop=(h == HPC - 1),
                                )
                    # Full-width [128, 2048] out tiles: 4KB DMA rows move the
                    # out stream from ~2KB descriptors to 4KB ones, roughly
                    # doubling effective write bandwidth so the final drain
                    # doesn't trail the last matmul by ~13us.
                    last = st == NT - 1
                    ot = opool.tile([128, 2048], BF16, name="ot", tag="ot", bufs=4)
                    nc.vector.tensor_copy(ot[:, 0:1024], pos[0][:])
                    nc.scalar.copy(ot[:, 1024:2048], pos[1][:])
                    if last:
                        # final tile: drain the two halves on BOTH DGE rings
                        # in parallel to halve the tail transfer
                        nc.scalar.dma_start(
                            out[st * 128 : (st + 1) * 128, 0:1024], ot[:, 0:1024]
                        )
                        nc.sync.dma_start(
                            out[st * 128 : (st + 1) * 128, 1024:2048], ot[:, 1024:2048]
                        )
                    else:
                        (nc.scalar if st % 2 == 0 else nc.sync).dma_start(
                            out[st * 128 : (st + 1) * 128, :], ot[:]
                        )

            if loop_ctx is not None:
                loop_ctx.__exit__(None, None, None)

    return nc


_NC = None


def _get_nc():
    global _NC
    if _NC is None:
        _NC = _build_bass()
    return _NC


def _prep_inputs(query, key, value, Wq, bq, Wk, bk, Wv, bv, Wo, bo):
    """Host-side shard + pack. Returns per-core input maps."""
    bf = ml_dtypes.bfloat16
    f32 = np.float32

    query = np.asarray(query, f32)
    key = np.asarray(key, f32)
    value = np.asarray(value, f32)
    Wq = np.asarray(Wq, f32)
    Wk = np.asarray(Wk, f32)
    Wv = np.asarray(Wv, f32)
    Wo = np.asarray(Wo, f32)
    bq = np.asarray(bq, f32)
    bk = np.asarray(bk, f32)
    bv = np.asarray(bv, f32)

    qT = np.ascontiguousarray(query.T).astype(bf)
    kT = np.ascontiguousarray(key.T).astype(bf)
    vT = np.ascontiguousarray(value.T).astype(bf)

    in_maps = []
    for c in range(N_CORES):
        heads = [c * HPC + j for j in range(HPC)]
        # wq/wk: [128, NT*HPC*128] strip-major: col = (dt*HPC + h)*128 + k,
        # row = d within tile (so a contiguous quarter = 4 strips, both heads)
        def pack_w(W):
            return np.concatenate(
                [
                    np.concatenate(
                        [W[hh].reshape(NT, 128, DK)[dt] for hh in heads], axis=1
                    )
                    for dt in range(NT)
                ],
                axis=1,
            ).astype(bf)

        # wv2: [128, NT*HPC*128], col = dt*(HPC*128) + h*128 + k
        wv2 = np.concatenate(
            [
                np.concatenate([Wv[hh].reshape(NT, 128, DK)[dt] for hh in heads], axis=1)
                for dt in range(NT)
            ],
            axis=1,
        ).astype(bf)

        wo_p = np.concatenate(
            [Wo[hh * DK : (hh + 1) * DK, :] for hh in heads], axis=1
        ).astype(bf)

        bqk = np.stack(
            [bq[hh] for hh in heads] + [bk[hh] for hh in heads], axis=1
        ).astype(f32)
        bvb = np.concatenate(
            [np.broadcast_to(bv[hh][None, :], (128, DK)) for hh in heads], axis=1
        ).astype(f32)

        in_maps.append(
            {
                "qT": qT,
                "kT": kT,
                "vT": vT,
                "wq": pack_w(Wq),
                "wk": pack_w(Wk),
                "wv2": np.ascontiguousarray(wv2),
                "wo": np.ascontiguousarray(wo_p),
                "bqk": np.ascontiguousarray(bqk),
                "bvb": np.ascontiguousarray(bvb),
            }
        )
    return in_maps


def kernel(query, key, value, Wq, bq, Wk, bk, Wv, bv, Wo, bo):
    global LAST_RESULTS
    in_maps = _prep_inputs(query, key, value, Wq, bq, Wk, bk, Wv, bv, Wo, bo)
    nc = _get_nc()
    res = bass_utils.run_bass_kernel_spmd(
        nc, in_maps, core_ids=list(range(N_CORES)), trace=TRACE
    )
    LAST_RESULTS = res
    acc = res.results[0]["out_p"].astype(np.float32)
    for c in range(1, N_CORES):
        acc += res.results[c]["out_p"].astype(np.float32)
    acc += np.asarray(bo, np.float32)[None, :]
    return acc



# revision 31
# speedup vs baseline: 1.0098x; 1.0098x over previous
"""Multi-head attention (softmax over the query axis) on 8 trn2 cores.

Sharding: tensor-parallel over heads — 2 heads per core. Each core computes
its heads' projections + attention + a partial output projection (row-parallel
Wo); the host sums the 8 partial outputs and adds bo.

Device-side layout choices (host pre-packs everything):
  - activations are shipped TRANSPOSED (d on partitions) as bf16, so every
    matmul contracts over the partition dim with natural-layout DMAs.
  - scores are computed transposed ([t, s]) so the softmax axis (query s) is
    the free axis: one exp-activation per strip with fused row-sum.
  - 1/rowsum is folded into V's rows (16K elems) instead of the attention
    matrix (4.2M elems).

Perf notes (measured on HW via NTFF profiles):
  - dependency-free warm-up matmuls bridge the initial DMA ramp so the PE's
    HAM clock gate reaches 2.4 GHz before real work starts (idle >3.4us
    re-throttles the array to 1.2 GHz).
  - consecutive duplicate Ldweights are elided at BIR level (bass emits one
    per matmul; the array retains the stationary operand).
  - output leaves in [128, 2048] bf16 tiles (4KB DMA rows) split across both
    HWDGE rings; the final tile drains half-per-ring.
  - O-phase loops h-outer so each head's HT strip loads once per st; a few
    pass2-h0 strips run early (inside PV+S0) to keep the ACT exp stream
    ahead of pass2-h1's weight loads.
"""

import json

import numpy as np
import ml_dtypes

import concourse.bass as bass
import concourse.mybir as mybir
import concourse.tile as tile
from concourse import bass_utils

BF16 = mybir.dt.bfloat16
F32 = mybir.dt.float32
AF = mybir.ActivationFunctionType

N_CORES = 8
H = 16
D = 2048
DK = 128
S = 2048
HPC = H // N_CORES          # heads per core = 2
NT = D // 128               # 16 tiles along d / t
NSC = S // 512              # 4 chunks of 512 along s / m
SCALE = 1.0 / float(np.sqrt(DK))

TRACE = False
LAST_RESULTS = None
PHASE_MARKS = []


def _mark(nc, label):
    PHASE_MARKS.append((label, nc.next_id()))


# The walrus in this container accepts only ONE sem-wait per instruction
# (setupSyncWait: "Too many sync wait commands"), but Tile attaches one wait
# per depended-on semaphore. Split extra waits onto single-wait NoOps inserted
# just before the instruction on the same engine, at BIR-JSON level so every
# compile path (native + bass2jax/axon) is covered.
def _split_multi_waits(raw: bytes) -> bytes:
    m = json.loads(raw)
    ctr = 0
    changed = False
    for fn in m.get("functions", []):
        for blk in fn.get("blocks", []):
            insts = blk.get("instructions", [])
            out = []
            for inst in insts:
                si = inst.get("sync_info")
                waits = (si.get("on_wait") or []) if si else []
                if len(waits) > 1:
                    changed = True
                    for w in waits[:-1]:
                        ctr += 1
                        out.append(
                            {
                                "debug": inst.get("debug"),
                                "engine": inst["engine"],
                                "ins": [],
                                "name": f"I-wsplit-{ctr}",
                                "opcode": "NoOp",
                                "outs": [],
                                "sync_info": {"on_update": [], "on_wait": [w]},
                            }
                        )
                    si["on_wait"] = [waits[-1]]
                out.append(inst)
            if changed:
                blk["instructions"] = out
    if not changed:
        return raw
    return json.dumps(m).encode()


# PE weight loads serialize with matmuls (~53ns each with FWL), and bass
# emits one Ldweights per Matmult even when consecutive matmuls share the
# same stationary operand (the array retains weights between matmuls — this
# is exactly what walrus's disabled --enable-ldw-opt pass would do, but that
# pass rejects pre-split Ldweights). Elide consecutive duplicates at BIR
# level, merging any waits into the paired Matmult (all updates already live
# on the Matmults; verified: every Ldweights is immediately followed by its
# Matmult in the scheduled stream).
def _elide_dup_ldweights(m: dict) -> None:
    for fn in m.get("functions", []):
        for blk in fn.get("blocks", []):
            insts = blk.get("instructions", [])
            out = []
            loaded = None  # weights key currently in the PE array
            pend_waits = None  # waits from an elided Ldweights
            for inst in insts:
                if inst.get("engine") != "PE":
                    out.append(inst)
                    continue
                op = inst.get("opcode")
                if op == "Ldweights":
                    key = json.dumps(
                        [
                            inst.get("ins"),
                            inst.get("tile_position"),
                            inst.get("tile_size"),
                            inst.get("perf_mode"),
                            inst.get("is_transpose"),
                        ],
                        sort_keys=True,
                    )
                    si = inst.get("sync_info") or {}
                    if key == loaded and not (si.get("on_update") or []):
                        w = si.get("on_wait") or []
                        if w:
                            pend_waits = (pend_waits or []) + w
                        continue  # drop the duplicate load
                    loaded = key
                    out.append(inst)
                elif op == "Matmult":
                    if pend_waits:
                        si = inst.setdefault("sync_info", {})
                        si["on_wait"] = (si.get("on_wait") or []) + pend_waits
                        pend_waits = None
                    out.append(inst)
                else:
                    # branches/drains/etc: conservatively forget array state
                    loaded = None
                    out.append(inst)
            blk["instructions"] = out


_orig_to_json_bytes = bass.Bass.to_json_bytes


def _to_json_bytes_split(self):
    m = json.loads(_orig_to_json_bytes(self))
    _elide_dup_ldweights(m)
    return _split_multi_waits(json.dumps(m).encode())


bass.Bass.to_json_bytes = _to_json_bytes_split


def _build_bass(loop_n=None):
    nc = bass.Bass(trn_type="TRN2")

    qT = nc.dram_tensor("qT", [D, S], BF16, kind="ExternalInput")
    kT = nc.dram_tensor("kT", [D, S], BF16, kind="ExternalInput")
    vT = nc.dram_tensor("vT", [D, S], BF16, kind="ExternalInput")
    wq = nc.dram_tensor("wq", [128, HPC * NT * 128], BF16, kind="ExternalInput")
    wk = nc.dram_tensor("wk", [128, HPC * NT * 128], BF16, kind="ExternalInput")
    wv2 = nc.dram_tensor("wv2", [128, NT * HPC * 128], BF16, kind="ExternalInput")
    wo = nc.dram_tensor("wo", [128, HPC * D], BF16, kind="ExternalInput")
    bqk = nc.dram_tensor("bqk", [128, 2 * HPC], F32, kind="ExternalInput")
    bvb = nc.dram_tensor("bvb", [128, HPC * 128], F32, kind="ExternalInput")
    out = nc.dram_tensor("out_p", [S, D], BF16, kind="ExternalOutput")

    with tile.TileContext(nc) as tc:
        with (
            tc.tile_pool(name="wpool", bufs=1) as wpool,
            tc.tile_pool(name="acts", bufs=1) as acts,
            tc.tile_pool(name="xpool", bufs=4) as xpool,
            tc.tile_pool(name="small", bufs=2) as small,
            tc.tile_pool(name="opool", bufs=2) as opool,
            tc.tile_pool(name="exppool", bufs=1) as exppool,
        ):
            # --- resident weights ---
            wq_sb = wpool.tile([128, HPC * NT * 128], BF16)
            wk_sb = wpool.tile([128, HPC * NT * 128], BF16)
            wv2_sb = wpool.tile([128, NT * HPC * 128], BF16)
            wo_sb = wpool.tile([128, HPC * D], BF16)
            bqk_sb = wpool.tile([128, 2 * HPC], F32)
            bvb_sb = wpool.tile([128, HPC * 128], F32)

            # --- resident per-head activations ---
            QT = [acts.tile([128, S], BF16, name=f"QT{h}") for h in range(HPC)]
            KT = [acts.tile([128, S], BF16, name=f"KT{h}") for h in range(HPC)]
            V = [acts.tile([128, NT * 128], BF16, name=f"V{h}") for h in range(HPC)]
            HT = [acts.tile([128, S], BF16, name=f"HT{h}") for h in range(HPC)]

            # Activation strips stream full-width (4KB rows -> fewest DMA
            # descriptors) on the SP DGE queue; ALL weight loads go through
            # the Activation DGE queue so their triggers issue in parallel
            # with (never behind) the strip stream. The 16 hardware DMA
            # queues are shared, but descriptor generation is per-engine.
            # wq/wk are packed strip-major (col = (dt*HPC+h)*128); wq is
            # DMA'd in three chunks so the first matmuls only wait on the
            # dt=0 strips, and the first activation strip lands in halves.
            xs_first = xpool.tile([128, S], BF16, name="xs", tag="xs", bufs=8)
            nc.sync.dma_start(xs_first[:, 0:1024], qT[0:128, 0:1024])
            nc.sync.dma_start(xs_first[:, 1024:2048], qT[0:128, 1024:2048])
            WQQ = HPC * NT * 32
            nc.scalar.dma_start(wq_sb[:, 0:256], wq[:, 0:256])
            nc.scalar.dma_start(wq_sb[:, 256:WQQ], wq[:, 256:WQQ])
            nc.scalar.dma_start(bqk_sb[:], bqk[:])

            # The PE's HAM clock gate keeps the array at 1.2 GHz until it has
            # been busy ~3.4us, and the initial DMA wait (~7us) resets that
            # window -- the whole first phase then runs at half clock. Bridge
            # the ramp with dependency-free warm-up matmuls on scratch data so
            # the PE is warm (2.4 GHz) the moment real operands land.
            with (
                tc.tile_pool(name="wusb", bufs=1) as wusb,
                tc.tile_pool(name="wups", bufs=1, space="PSUM") as wups,
            ):
                wsrc = wusb.tile([128, 256], BF16)
                nc.vector.memset(wsrc[:], 0.0)
                wp = wups.tile([128, 256], F32)
                NWU = 24
                for i in range(NWU):
                    nc.tensor.matmul(
                        wp[:], wsrc[:, 0:128], wsrc[:], start=(i == 0), stop=(i == NWU - 1)
                    )

            # benchmark mode: run the whole body loop_n times in one NEFF to
            # amortize dispatch overhead; weights loaded once up front.
            loop_ctx = None
            if loop_n:
                nc.sync.dma_start(wk_sb[:], wk[:])
                nc.sync.dma_start(wv2_sb[:], wv2[:])
                nc.sync.dma_start(bvb_sb[:], bvb[:])
                nc.sync.dma_start(wo_sb[:], wo[:])
                loop_ctx = tc.For_i(0, loop_n, 1)
                loop_ctx.__enter__()

            # ---------------- phase P-QK: Q^T / K^T projections ---------------
            # Full-strip loads ([128, S] = 4KB lines); one psum bank per
            # (head, s-chunk), accumulated across all 16 d-strips.
            with tc.tile_pool(name="ppqk", bufs=1, space="PSUM") as ppqk:
                for xdram, w_sb, dst, bcol in ((qT, wq_sb, QT, 0), (kT, wk_sb, KT, HPC)):
                    _mark(nc, "P-Q" if xdram is qT else "P-K")
                    if xdram is kT and not loop_n:
                        # on the SP queue: paced behind the q-strip triggers
                        # (their buffer-reuse waits), so it cannot crowd out
                        # the strip stream during the DMA ramp
                        nc.sync.dma_start(wk_sb[:], wk[:])
                    ps = [
                        [
                            ppqk.tile(
                                [128, 512], F32, name=f"pp{h}{c}", tag=f"pp{h}{c}", bufs=1
                            )
                            for c in range(NSC)
                        ]
                        for h in range(HPC)
                    ]
                    for dt in range(NT):
                        if xdram is qT and dt == 0:
                            xs = xs_first
                        else:
                            xs = xpool.tile([128, S], BF16, name="xs", tag="xs", bufs=8)
                            nc.sync.dma_start(xs[:], xdram[dt * 128 : (dt + 1) * 128, :])
                        if xdram is qT and not loop_n:
                            # wv2/bvb on the SP queue mid-loop: the strip
                            # triggers ahead of them carry buffer-reuse waits,
                            # so these fire mid-phase (after the DMA ramp)
                            # instead of crowding the ramp like the ungated
                            # Activation DGE queue would
                            if dt == 3:
                                # bulk of wq rides the strip ring BEHIND the
                                # first strips: the ramp delivers strips at
                                # full bandwidth, and this still lands well
                                # before the dt=4 matmuls need it
                                nc.sync.dma_start(
                                    wq_sb[:, WQQ : 4 * WQQ], wq[:, WQQ : 4 * WQQ]
                                )
                            if dt == 6:
                                nc.sync.dma_start(
                                    wv2_sb[:, 0 : NT * HPC * 64],
                                    wv2[:, 0 : NT * HPC * 64],
                                )
                            elif dt == 8:
                                nc.sync.dma_start(
                                    wv2_sb[:, NT * HPC * 64 : NT * HPC * 128],
                                    wv2[:, NT * HPC * 64 : NT * HPC * 128],
                                )
                            elif dt == 10:
                                nc.sync.dma_start(bvb_sb[:], bvb[:])
                        for h in range(HPC):
                            for c in range(NSC):
                                nc.tensor.matmul(
                                    ps[h][c][:],
                                    w_sb[:, (dt * HPC + h) * 128 : (dt * HPC + h + 1) * 128],
                                    xs[:, c * 512 : (c + 1) * 512],
                                    start=(dt == 0),
                                    stop=(dt == NT - 1),
                                )
                    # psum -> sbuf with bias, in bank order with alternating
                    # engines so every bank drains (and is reusable by the
                    # next phase) shortly after its own last matmul.
                    # (GpSimd/Pool cannot read PSUM — compile-rejected.)
                    for h in range(HPC):
                        for c in range(NSC):
                            if c % 2 == 0:
                                nc.scalar.activation(
                                    dst[h][:, c * 512 : (c + 1) * 512],
                                    ps[h][c][:],
                                    AF.Identity,
                                    bias=bqk_sb[:, bcol + h : bcol + h + 1],
                                    scale=1.0,
                                )
                            else:
                                nc.vector.tensor_scalar_add(
                                    dst[h][:, c * 512 : (c + 1) * 512],
                                    ps[h][c][:],
                                    bqk_sb[:, bcol + h : bcol + h + 1],
                                )

            # ------- phases P-V and S interleaved on one psum pool -----------
            # P-V shares the ph* psum tags (1 bank each) with S pass 2; the V
            # matmul groups are interleaved with S-h0 pass 1 so the PE stays
            # busy under pass 1's ACT-bound exp stream.
            with tc.tile_pool(name="pps", bufs=1, space="PSUM") as pps:
                if not loop_n:
                    nc.scalar.dma_start(wo_sb[:], wo[:])
                vsca = [
                    small.tile([128, NT * 128], BF16, name=f"vsca{h}", tag=f"vsca{h}", bufs=1)
                    for h in range(HPC)
                ]
                expts = {}

                def emit_v_group(tg):
                    # V: [t, hk] natural layout, both heads fused per matmul
                    psv = [
                        pps.tile(
                            [128, HPC * 128], F32, name=f"psv{tt}", tag=f"ph{tt}", bufs=1
                        )
                        for tt in range(4)
                    ]
                    for dt in range(NT):
                        xc = xpool.tile([128, 512], BF16, name="xc", tag="xc", bufs=12)
                        nc.sync.dma_start(
                            xc[:], vT[dt * 128 : (dt + 1) * 128, tg * 512 : (tg + 1) * 512]
                        )
                        for tt in range(4):
                            nc.tensor.matmul(
                                psv[tt][:],
                                xc[:, tt * 128 : (tt + 1) * 128],
                                wv2_sb[:, dt * HPC * 128 : (dt + 1) * HPC * 128],
                                start=(dt == 0),
                                stop=(dt == NT - 1),
                            )
                    for tt in range(4):
                        t_tile = tg * 4 + tt
                        for h in range(HPC):
                            nc.vector.tensor_tensor(
                                V[h][:, t_tile * 128 : (t_tile + 1) * 128],
                                psv[tt][:, h * 128 : (h + 1) * 128],
                                bvb_sb[:, h * 128 : (h + 1) * 128],
                                op=mybir.AluOpType.add,
                            )

                rects = {}

                def emit_pass1_partA(h, i):
                    # scores -> exp (+row-sum) -> 1/rowsum. h1's first strips
                    # run early (inside PV+S0, which has ACT slack) and need
                    # their own buffers -- the exp{i} buffer still holds h0's
                    # strip until pass2-h0 consumes it.
                    tag = f"expb{i}" if (h == 1 and i < HOIST) else f"exp{i}"
                    expt = exppool.tile([128, S], BF16, name=f"expt{i}", tag=tag, bufs=1)
                    expts[(h, i)] = expt
                    sumt = small.tile([128, 2], F32, name="sumt", tag="sum", bufs=4)
                    for half in range(2):
                        psc = pps.tile(
                            [128, 1024], F32, name=f"psc{half}", tag=f"psc{half}", bufs=1
                        )
                        for cc in range(2):
                            c = half * 2 + cc
                            nc.tensor.matmul(
                                psc[:, cc * 512 : (cc + 1) * 512],
                                KT[h][:, i * 128 : (i + 1) * 128],
                                QT[h][:, c * 512 : (c + 1) * 512],
                                start=True,
                                stop=True,
                            )
                        nc.scalar.activation(
                            expt[:, half * 1024 : (half + 1) * 1024],
                            psc[:],
                            AF.Exp,
                            scale=SCALE,
                            accum_out=sumt[:, half : half + 1],
                        )
                    rect = small.tile(
                        [128, 1], F32, name="rect", tag=f"rec{i % 4}", bufs=2
                    )
                    rects[(h, i)] = rect
                    nc.vector.reduce_sum(rect[:], sumt[:], axis=mybir.AxisListType.X)
                    nc.vector.reciprocal(rect[:], rect[:])

                def emit_pass1_partB(h, i):
                    # fold 1/rowsum into this strip's V rows
                    nc.vector.tensor_scalar_mul(
                        vsca[h][:, i * 128 : (i + 1) * 128],
                        V[h][:, i * 128 : (i + 1) * 128],
                        rects[(h, i)][:],
                    )

                def emit_pass1_strip(h, i):
                    emit_pass1_partA(h, i)
                    emit_pass1_partB(h, i)

                def emit_pass2_strip(h, ph, i):
                    for c in range(NSC):
                        nc.tensor.matmul(
                            ph[c][:],
                            vsca[h][:, i * 128 : (i + 1) * 128],
                            expts[(h, i)][:, c * 512 : (c + 1) * 512],
                            start=(i == 0),
                            stop=(i == NT - 1),
                        )

                def emit_ht_copies(h, ph):
                    # h0's copies run while the Scalar engine is still
                    # saturated with h1's exp stream -> keep them off Scalar.
                    # h1's copies are on the O-phase critical path and the exp
                    # stream is done -> split across both engines.
                    for c in range(NSC):
                        if h == 1 and c % 2 == 1:
                            nc.scalar.copy(
                                HT[h][:, c * 512 : (c + 1) * 512], ph[c][:]
                            )
                        else:
                            nc.vector.tensor_copy(
                                HT[h][:, c * 512 : (c + 1) * 512], ph[c][:]
                            )

                def emit_pass2(h):
                    ph = [
                        pps.tile([128, 512], F32, name=f"ph{c}", tag=f"ph{c}", bufs=1)
                        for c in range(NSC)
                    ]
                    for i in range(NT):
                        emit_pass2_strip(h, ph, i)
                    emit_ht_copies(h, ph)

                _mark(nc, "PV+S0")
                # Interleave: strips' scores+exp first (feed ACT), V matmul
                # group fills PE during the exps, then the vsc muls that need
                # this group's V tiles. The last groups also hoist h1's first
                # exp strips into this phase's ACT slack so the S2h0+S1h1
                # phase is PE-bound instead of gating S2-h1 on the exp tail.
                HOIST = 2
                rects = {}
                for g in range(4):
                    for i in range(4 * g, 4 * g + 4):
                        emit_pass1_partA(0, i)
                    if g >= 2:
                        emit_pass1_partA(1, g - 2)
                    emit_v_group(g)
                    for i in range(4 * g, 4 * g + 4):
                        emit_pass1_partB(0, i)
                    if g >= 2:
                        emit_pass1_partB(1, g - 2)
                # a few pass2-h0 strips run at the tail of PV+S0 (PE-cheap
                # there) so the S2h0+S1h1 phase has less PE work than ACT
                # work -- otherwise the h1 exp stream finishes after pass2-h0
                # and S2-h1's first weight loads stall on it
                EARLY = 3
                ph0 = [
                    pps.tile([128, 512], F32, name=f"ph{c}", tag=f"ph{c}", bufs=1)
                    for c in range(NSC)
                ]
                for i in range(EARLY):
                    emit_pass2_strip(0, ph0, i)
                _mark(nc, "S2h0+S1h1")
                # pass2 of head 0 (PE-dense) strip-interleaved with pass1 of
                # head 1 (ACT-bound)
                for i in range(NT):
                    if i >= EARLY:
                        emit_pass2_strip(0, ph0, i)
                    if i >= HOIST:
                        emit_pass1_partA(1, i)
                        emit_pass1_partB(1, i)
                emit_ht_copies(0, ph0)
                _mark(nc, "S2-h1")
                emit_pass2(1)

            # ---------------- phase O: partial output projection --------------
            with tc.tile_pool(name="ppo", bufs=4, space="PSUM") as ppo:
                _mark(nc, "O")
                for st in range(NT):
                    # h-outer so each head's stationary HT strip is loaded
                    # once per st (the Ldweights-dedup BIR pass elides the
                    # repeats) instead of alternating HT0/HT1 every matmul.
                    pos = [
                        ppo.tile([128, 1024], F32, name="po", tag="po", bufs=4)
                        for _ in range(2)
                    ]
                    for h in range(HPC):
                        for cp in range(2):
                            for cc in range(2):
                                c = cp * 2 + cc
                                nc.tensor.matmul(
                                    pos[cp][:, cc * 512 : (cc + 1) * 512],
                                    HT[h][:, st * 128 : (st + 1) * 128],
                                    wo_sb[:, h * D + c * 512 : h * D + (c + 1) * 512],
                                    start=(h == 0),
                                    stop=(h == HPC - 1),
                                )
                    # Full-width [128, 2048] out tiles: 4KB DMA rows move the
                    # out stream from ~2KB descriptors to 4KB ones, roughly
                    # doubling effective write bandwidth so the final drain
                    # doesn't trail the last matmul by ~13us.
                    last = st == NT - 1
                    ot = opool.tile([128, 2048], BF16, name="ot", tag="ot", bufs=4)
                    nc.vector.tensor_copy(ot[:, 0:1024], pos[0][:])
                    nc.scalar.copy(ot[:, 1024:2048], pos[1][:])
                    if last:
                        # final tile: drain the two halves on BOTH DGE rings
                        # in parallel to halve the tail transfer
                        nc.scalar.dma_start(
                            out[st * 128 : (st + 1) * 128, 0:1024], ot[:, 0:1024]
                        )
                        nc.sync.dma_start(
                            out[st * 128 : (st + 1) * 128, 1024:2048], ot[:, 1024:2048]
                        )
                    else:
                        (nc.scalar if st % 2 == 0 else nc.sync).dma_start(
                            out[st * 128 : (st + 1) * 128, :], ot[:]
                        )

            if loop_ctx is not None:
                loop_ctx.__exit__(None, None, None)

    return nc


_NC = None


def _get_nc():
    global _NC
    if _NC is None:
        _NC = _build_bass()
    return _NC


def _prep_inputs(query, key, value, Wq, bq, Wk, bk, Wv, bv, Wo, bo):
    """Host-side shard + pack. Returns per-core input maps."""
    bf = ml_dtypes.bfloat16
    f32 = np.float32

    query = np.asarray(query, f32)
    key = np.asarray(key, f32)
    value = np.asarray(value, f32)
    Wq = np.asarray(Wq, f32)
    Wk = np.asarray(Wk, f32)
    Wv = np.asarray(Wv, f32)
    Wo = np.asarray(Wo, f32)
    bq = np.asarray(bq, f32)
    bk = np.asarray(bk, f32)
    bv = np.asarray(bv, f32)

    qT = np.ascontiguousarray(query.T).astype(bf)
    kT = np.ascontiguousarray(key.T).astype(bf)
    vT = np.ascontiguousarray(value.T).astype(bf)

    in_maps = []
    for c in range(N_CORES):
        heads = [c * HPC + j for j in range(HPC)]
        # wq/wk: [128, NT*HPC*128] strip-major: col = (dt*HPC + h)*128 + k,
        # row = d within tile (so a contiguous quarter = 4 strips, both heads)
        def pack_w(W):
            return np.concatenate(
                [
                    np.concatenate(
                        [W[hh].reshape(NT, 128, DK)[dt] for hh in heads], axis=1
                    )
                    for dt in range(NT)
                ],
                axis=1,
            ).astype(bf)

        # wv2: [128, NT*HPC*128], col = dt*(HPC*128) + h*128 + k
        wv2 = np.concatenate(
            [
                np.concatenate([Wv[hh].reshape(NT, 128, DK)[dt] for hh in heads], axis=1)
                for dt in range(NT)
            ],
            axis=1,
        ).astype(bf)

        wo_p = np.concatenate(
            [Wo[hh * DK : (hh + 1) * DK, :] for hh in heads], axis=1
        ).astype(bf)

        bqk = np.stack(
            [bq[hh] for hh in heads] + [bk[hh] for hh in heads], axis=1
        ).astype(f32)
        bvb = np.concatenate(
            [np.broadcast_to(bv[hh][None, :], (128, DK)) for hh in heads], axis=1
        ).astype(f32)

        in_maps.append(
            {
                "qT": qT,
                "kT": kT,
                "vT": vT,
                "wq": pack_w(Wq),
                "wk": pack_w(Wk),
                "wv2": np.ascontiguousarray(wv2),
                "wo": np.ascontiguousarray(wo_p),
                "bqk": np.ascontiguousarray(bqk),
                "bvb": np.ascontiguousarray(bvb),
            }
        )
    return in_maps


def kernel(query, key, value, Wq, bq, Wk, bk, Wv, bv, Wo, bo):
    global LAST_RESULTS
    in_maps = _prep_inputs(query, key, value, Wq, bq, Wk, bk, Wv, bv, Wo, bo)
    nc = _get_nc()
    res = bass_utils.run_bass_kernel_spmd(
        nc, in_maps, core_ids=list(range(N_CORES)), trace=TRACE
    )
    LAST_RESULTS = res
    acc = res.results[0]["out_p"].astype(np.float32)
    for c in range(1, N_CORES):
        acc += res.results[c]["out_p"].astype(np.float32)
    acc += np.asarray(bo, np.float32)[None, :]
    return acc



# revision 33
# speedup vs baseline: 1.2045x; 1.1928x over previous
"""Multi-head attention (softmax over the query axis) on 8 trn2 cores.

Sharding: tensor-parallel over heads — 2 heads per core. Each core computes
its heads' projections + attention + a partial output projection (row-parallel
Wo); the host sums the 8 partial outputs and adds bo.

Device-side layout choices (host pre-packs everything):
  - activations are shipped TRANSPOSED (d on partitions) as bf16, so every
    matmul contracts over the partition dim with natural-layout DMAs.
  - scores are computed transposed ([t, s]) so the softmax axis (query s) is
    the free axis: one exp-activation per strip with fused row-sum.
  - 1/rowsum is folded into V's rows (16K elems) instead of the attention
    matrix (4.2M elems).

Perf notes (measured on HW via NTFF profiles):
  - dependency-free warm-up matmuls bridge the initial DMA ramp so the PE's
    HAM clock gate reaches 2.4 GHz before real work starts (idle >3.4us
    re-throttles the array to 1.2 GHz).
  - consecutive duplicate Ldweights are elided at BIR level (bass emits one
    per matmul; the array retains the stationary operand).
  - output leaves in [128, 2048] bf16 tiles (4KB DMA rows) split across both
    HWDGE rings; the final tile drains half-per-ring.
  - O-phase loops h-outer so each head's HT strip loads once per st; a few
    pass2-h0 strips run early (inside PV+S0) to keep the ACT exp stream
    ahead of pass2-h1's weight loads.
"""

import json

import numpy as np
import ml_dtypes

import concourse.bass as bass
import concourse.mybir as mybir
import concourse.tile as tile
from concourse import bass_utils

BF16 = mybir.dt.bfloat16
F32 = mybir.dt.float32
AF = mybir.ActivationFunctionType

N_CORES = 8
H = 16
D = 2048
DK = 128
S = 2048
HPC = H // N_CORES          # heads per core = 2
NT = D // 128               # 16 tiles along d / t
NSC = S // 512              # 4 chunks of 512 along s / m
SCALE = 1.0 / float(np.sqrt(DK))

TRACE = False
LAST_RESULTS = None
PHASE_MARKS = []


def _mark(nc, label):
    PHASE_MARKS.append((label, nc.next_id()))


# The walrus in this container accepts only ONE sem-wait per instruction
# (setupSyncWait: "Too many sync wait commands"), but Tile attaches one wait
# per depended-on semaphore. Split extra waits onto single-wait NoOps inserted
# just before the instruction on the same engine, at BIR-JSON level so every
# compile path (native + bass2jax/axon) is covered.
def _split_multi_waits(raw: bytes) -> bytes:
    m = json.loads(raw)
    ctr = 0
    changed = False
    for fn in m.get("functions", []):
        for blk in fn.get("blocks", []):
            insts = blk.get("instructions", [])
            out = []
            for inst in insts:
                si = inst.get("sync_info")
                waits = (si.get("on_wait") or []) if si else []
                if len(waits) > 1:
                    changed = True
                    for w in waits[:-1]:
                        ctr += 1
                        out.append(
                            {
                                "debug": inst.get("debug"),
                                "engine": inst["engine"],
                                "ins": [],
                                "name": f"I-wsplit-{ctr}",
                                "opcode": "NoOp",
                                "outs": [],
                                "sync_info": {"on_update": [], "on_wait": [w]},
                            }
                        )
                    si["on_wait"] = [waits[-1]]
                out.append(inst)
            if changed:
                blk["instructions"] = out
    if not changed:
        return raw
    return json.dumps(m).encode()


# PE weight loads serialize with matmuls (~53ns each with FWL), and bass
# emits one Ldweights per Matmult even when consecutive matmuls share the
# same stationary operand (the array retains weights between matmuls — this
# is exactly what walrus's disabled --enable-ldw-opt pass would do, but that
# pass rejects pre-split Ldweights). Elide consecutive duplicates at BIR
# level, merging any waits into the paired Matmult (all updates already live
# on the Matmults; verified: every Ldweights is immediately followed by its
# Matmult in the scheduled stream).
def _elide_dup_ldweights(m: dict) -> None:
    for fn in m.get("functions", []):
        for blk in fn.get("blocks", []):
            insts = blk.get("instructions", [])
            out = []
            loaded = None  # weights key currently in the PE array
            pend_waits = None  # waits from an elided Ldweights
            for inst in insts:
                if inst.get("engine") != "PE":
                    out.append(inst)
                    continue
                op = inst.get("opcode")
                if op == "Ldweights":
                    key = json.dumps(
                        [
                            inst.get("ins"),
                            inst.get("tile_position"),
                            inst.get("tile_size"),
                            inst.get("perf_mode"),
                            inst.get("is_transpose"),
                        ],
                        sort_keys=True,
                    )
                    si = inst.get("sync_info") or {}
                    if key == loaded and not (si.get("on_update") or []):
                        w = si.get("on_wait") or []
                        if w:
                            pend_waits = (pend_waits or []) + w
                        continue  # drop the duplicate load
                    loaded = key
                    out.append(inst)
                elif op == "Matmult":
                    if pend_waits:
                        si = inst.setdefault("sync_info", {})
                        si["on_wait"] = (si.get("on_wait") or []) + pend_waits
                        pend_waits = None
                    out.append(inst)
                else:
                    # branches/drains/etc: conservatively forget array state
                    loaded = None
                    out.append(inst)
            blk["instructions"] = out


_orig_to_json_bytes = bass.Bass.to_json_bytes


def _to_json_bytes_split(self):
    m = json.loads(_orig_to_json_bytes(self))
    _elide_dup_ldweights(m)
    return _split_multi_waits(json.dumps(m).encode())


bass.Bass.to_json_bytes = _to_json_bytes_split


def _build_bass(loop_n=None):
    nc = bass.Bass(trn_type="TRN2")

    qT = nc.dram_tensor("qT", [D, S], BF16, kind="ExternalInput")
    kT = nc.dram_tensor("kT", [D, S], BF16, kind="ExternalInput")
    vT = nc.dram_tensor("vT", [D, S], BF16, kind="ExternalInput")
    wq = nc.dram_tensor("wq", [128, HPC * NT * 128], BF16, kind="ExternalInput")
    wk = nc.dram_tensor("wk", [128, HPC * NT * 128], BF16, kind="ExternalInput")
    wv2 = nc.dram_tensor("wv2", [128, NT * HPC * 128], BF16, kind="ExternalInput")
    wo = nc.dram_tensor("wo", [128, HPC * D], BF16, kind="ExternalInput")
    bqk = nc.dram_tensor("bqk", [128, 2 * HPC], F32, kind="ExternalInput")
    bvb = nc.dram_tensor("bvb", [128, HPC * 128], F32, kind="ExternalInput")
    out = nc.dram_tensor("out_p", [S, D], BF16, kind="ExternalOutput")

    with tile.TileContext(nc) as tc:
        with (
            tc.tile_pool(name="wpool", bufs=1) as wpool,
            tc.tile_pool(name="acts", bufs=1) as acts,
            tc.tile_pool(name="xpool", bufs=4) as xpool,
            tc.tile_pool(name="small", bufs=2) as small,
            tc.tile_pool(name="opool", bufs=2) as opool,
            tc.tile_pool(name="exppool", bufs=1) as exppool,
        ):
            # --- resident weights ---
            wq_sb = wpool.tile([128, HPC * NT * 128], BF16)
            wk_sb = wpool.tile([128, HPC * NT * 128], BF16)
            wv2_sb = wpool.tile([128, NT * HPC * 128], BF16)
            wo_sb = wpool.tile([128, HPC * D], BF16)
            bqk_sb = wpool.tile([128, 2 * HPC], F32)
            bvb_sb = wpool.tile([128, HPC * 128], F32)

            # --- resident per-head activations ---
            QT = [acts.tile([128, S], BF16, name=f"QT{h}") for h in range(HPC)]
            KT = [acts.tile([128, S], BF16, name=f"KT{h}") for h in range(HPC)]
            V = [acts.tile([128, NT * 128], BF16, name=f"V{h}") for h in range(HPC)]
            HT = [acts.tile([128, S], BF16, name=f"HT{h}") for h in range(HPC)]

            # Activation strips stream full-width (4KB rows -> fewest DMA
            # descriptors) on the SP DGE queue; ALL weight loads go through
            # the Activation DGE queue so their triggers issue in parallel
            # with (never behind) the strip stream. The 16 hardware DMA
            # queues are shared, but descriptor generation is per-engine.
            # wq/wk are packed strip-major (col = (dt*HPC+h)*128); wq is
            # DMA'd in three chunks so the first matmuls only wait on the
            # dt=0 strips, and the first activation strip lands in halves.
            xs_first = xpool.tile([128, S], BF16, name="xs", tag="xs", bufs=8)
            nc.sync.dma_start(xs_first[:, 0:1024], qT[0:128, 0:1024])
            nc.sync.dma_start(xs_first[:, 1024:2048], qT[0:128, 1024:2048])
            WQQ = HPC * NT * 32
            nc.scalar.dma_start(wq_sb[:, 0:256], wq[:, 0:256])
            nc.scalar.dma_start(wq_sb[:, 256:WQQ], wq[:, 256:WQQ])
            nc.scalar.dma_start(bqk_sb[:], bqk[:])

            # The PE's HAM clock gate keeps the array at 1.2 GHz until it has
            # been busy ~3.4us, and the initial DMA wait (~7us) resets that
            # window -- the whole first phase then runs at half clock. Bridge
            # the ramp with dependency-free warm-up matmuls on scratch data so
            # the PE is warm (2.4 GHz) the moment real operands land.
            with (
                tc.tile_pool(name="wusb", bufs=1) as wusb,
                tc.tile_pool(name="wups", bufs=1, space="PSUM") as wups,
            ):
                wsrc = wusb.tile([128, 256], BF16)
                nc.vector.memset(wsrc[:], 0.0)
                wp = wups.tile([128, 256], F32)
                NWU = 24
                for i in range(NWU):
                    nc.tensor.matmul(
                        wp[:], wsrc[:, 0:128], wsrc[:], start=(i == 0), stop=(i == NWU - 1)
                    )

            # benchmark mode: run the whole body loop_n times in one NEFF to
            # amortize dispatch overhead; weights loaded once up front.
            loop_ctx = None
            if loop_n:
                nc.sync.dma_start(wk_sb[:], wk[:])
                nc.sync.dma_start(wv2_sb[:], wv2[:])
                nc.sync.dma_start(bvb_sb[:], bvb[:])
                nc.sync.dma_start(wo_sb[:], wo[:])
                loop_ctx = tc.For_i(0, loop_n, 1)
                loop_ctx.__enter__()

            # ---------------- phase P-QK: Q^T / K^T projections ---------------
            # Full-strip loads ([128, S] = 4KB lines); one psum bank per
            # (head, s-chunk), accumulated across all 16 d-strips.
            with tc.tile_pool(name="ppqk", bufs=1, space="PSUM") as ppqk:
                for xdram, w_sb, dst, bcol in ((qT, wq_sb, QT, 0), (kT, wk_sb, KT, HPC)):
                    _mark(nc, "P-Q" if xdram is qT else "P-K")
                    if xdram is kT and not loop_n:
                        # on the SP queue: paced behind the q-strip triggers
                        # (their buffer-reuse waits), so it cannot crowd out
                        # the strip stream during the DMA ramp
                        nc.sync.dma_start(wk_sb[:], wk[:])
                    ps = [
                        [
                            ppqk.tile(
                                [128, 512], F32, name=f"pp{h}{c}", tag=f"pp{h}{c}", bufs=1
                            )
                            for c in range(NSC)
                        ]
                        for h in range(HPC)
                    ]
                    for dt in range(NT):
                        if xdram is qT and dt == 0:
                            xs = xs_first
                        else:
                            xs = xpool.tile([128, S], BF16, name="xs", tag="xs", bufs=8)
                            nc.sync.dma_start(xs[:], xdram[dt * 128 : (dt + 1) * 128, :])
                        if xdram is qT and not loop_n:
                            # wv2/bvb on the SP queue mid-loop: the strip
                            # triggers ahead of them carry buffer-reuse waits,
                            # so these fire mid-phase (after the DMA ramp)
                            # instead of crowding the ramp like the ungated
                            # Activation DGE queue would
                            if dt == 3:
                                # bulk of wq rides the strip ring BEHIND the
                                # first strips: the ramp delivers strips at
                                # full bandwidth, and this still lands well
                                # before the dt=4 matmuls need it
                                nc.sync.dma_start(
                                    wq_sb[:, WQQ : 4 * WQQ], wq[:, WQQ : 4 * WQQ]
                                )
                            if dt == 6:
                                nc.sync.dma_start(
                                    wv2_sb[:, 0 : NT * HPC * 64],
                                    wv2[:, 0 : NT * HPC * 64],
                                )
                            elif dt == 8:
                                nc.sync.dma_start(
                                    wv2_sb[:, NT * HPC * 64 : NT * HPC * 128],
                                    wv2[:, NT * HPC * 64 : NT * HPC * 128],
                                )
                            elif dt == 10:
                                nc.sync.dma_start(bvb_sb[:], bvb[:])
                            elif dt == 12:
                                # wo on the sync ring mid-qT: a scalar-engine
                                # trigger would not fire until the QK bias
                                # drains finish (~80us), and the late
                                # completion head-of-line-blocks its shared
                                # DMA-completion lane for later consumers
                                nc.sync.dma_start(wo_sb[:], wo[:])
                        for h in range(HPC):
                            for c in range(NSC):
                                nc.tensor.matmul(
                                    ps[h][c][:],
                                    w_sb[:, (dt * HPC + h) * 128 : (dt * HPC + h + 1) * 128],
                                    xs[:, c * 512 : (c + 1) * 512],
                                    start=(dt == 0),
                                    stop=(dt == NT - 1),
                                )
                    # psum -> sbuf with bias, in bank order with alternating
                    # engines so every bank drains (and is reusable by the
                    # next phase) shortly after its own last matmul.
                    # (GpSimd/Pool cannot read PSUM — compile-rejected.)
                    for h in range(HPC):
                        for c in range(NSC):
                            if c % 2 == 0:
                                nc.scalar.activation(
                                    dst[h][:, c * 512 : (c + 1) * 512],
                                    ps[h][c][:],
                                    AF.Identity,
                                    bias=bqk_sb[:, bcol + h : bcol + h + 1],
                                    scale=1.0,
                                )
                            else:
                                nc.vector.tensor_scalar_add(
                                    dst[h][:, c * 512 : (c + 1) * 512],
                                    ps[h][c][:],
                                    bqk_sb[:, bcol + h : bcol + h + 1],
                                )

            # ------- phases P-V and S interleaved on one psum pool -----------
            # P-V shares the ph* psum tags (1 bank each) with S pass 2; the V
            # matmul groups are interleaved with S-h0 pass 1 so the PE stays
            # busy under pass 1's ACT-bound exp stream.
            with tc.tile_pool(name="pps", bufs=1, space="PSUM") as pps:
                vsca = [
                    small.tile([128, NT * 128], BF16, name=f"vsca{h}", tag=f"vsca{h}", bufs=1)
                    for h in range(HPC)
                ]
                expts = {}

                def emit_v_group(tg):
                    # V: [t, hk] natural layout, both heads fused per matmul
                    psv = [
                        pps.tile(
                            [128, HPC * 128], F32, name=f"psv{tt}", tag=f"ph{tt}", bufs=1
                        )
                        for tt in range(4)
                    ]
                    for dt in range(NT):
                        xc = xpool.tile([128, 512], BF16, name="xc", tag="xc", bufs=12)
                        nc.sync.dma_start(
                            xc[:], vT[dt * 128 : (dt + 1) * 128, tg * 512 : (tg + 1) * 512]
                        )
                        for tt in range(4):
                            nc.tensor.matmul(
                                psv[tt][:],
                                xc[:, tt * 128 : (tt + 1) * 128],
                                wv2_sb[:, dt * HPC * 128 : (dt + 1) * HPC * 128],
                                start=(dt == 0),
                                stop=(dt == NT - 1),
                            )
                    for tt in range(4):
                        t_tile = tg * 4 + tt
                        for h in range(HPC):
                            nc.vector.tensor_tensor(
                                V[h][:, t_tile * 128 : (t_tile + 1) * 128],
                                psv[tt][:, h * 128 : (h + 1) * 128],
                                bvb_sb[:, h * 128 : (h + 1) * 128],
                                op=mybir.AluOpType.add,
                            )

                rects = {}

                def emit_pass1_partA(h, i):
                    # scores -> exp (+row-sum) -> 1/rowsum. h1's first strips
                    # run early (inside PV+S0, which has ACT slack) and need
                    # their own buffers -- the exp{i} buffer still holds h0's
                    # strip until pass2-h0 consumes it.
                    tag = f"expb{i}" if (h == 1 and i < HOIST) else f"exp{i}"
                    expt = exppool.tile([128, S], BF16, name=f"expt{i}", tag=tag, bufs=1)
                    expts[(h, i)] = expt
                    sumt = small.tile([128, 2], F32, name="sumt", tag="sum", bufs=4)
                    for half in range(2):
                        psc = pps.tile(
                            [128, 1024], F32, name=f"psc{half}", tag=f"psc{half}", bufs=1
                        )
                        for cc in range(2):
                            c = half * 2 + cc
                            nc.tensor.matmul(
                                psc[:, cc * 512 : (cc + 1) * 512],
                                KT[h][:, i * 128 : (i + 1) * 128],
                                QT[h][:, c * 512 : (c + 1) * 512],
                                start=True,
                                stop=True,
                            )
                        nc.scalar.activation(
                            expt[:, half * 1024 : (half + 1) * 1024],
                            psc[:],
                            AF.Exp,
                            scale=SCALE,
                            accum_out=sumt[:, half : half + 1],
                        )
                    rect = small.tile(
                        [128, 1], F32, name="rect", tag=f"rec{i % 4}", bufs=2
                    )
                    rects[(h, i)] = rect
                    nc.vector.reduce_sum(rect[:], sumt[:], axis=mybir.AxisListType.X)
                    nc.vector.reciprocal(rect[:], rect[:])

                def emit_pass1_partB(h, i):
                    # fold 1/rowsum into this strip's V rows
                    nc.vector.tensor_scalar_mul(
                        vsca[h][:, i * 128 : (i + 1) * 128],
                        V[h][:, i * 128 : (i + 1) * 128],
                        rects[(h, i)][:],
                    )

                def emit_pass1_strip(h, i):
                    emit_pass1_partA(h, i)
                    emit_pass1_partB(h, i)

                def emit_pass2_strip(h, ph, i):
                    for c in range(NSC):
                        nc.tensor.matmul(
                            ph[c][:],
                            vsca[h][:, i * 128 : (i + 1) * 128],
                            expts[(h, i)][:, c * 512 : (c + 1) * 512],
                            start=(i == 0),
                            stop=(i == NT - 1),
                        )

                def emit_ht_copies(h, ph):
                    # h0's copies run while the Scalar engine is still
                    # saturated with h1's exp stream -> keep them off Scalar.
                    # h1's copies are on the O-phase critical path and the exp
                    # stream is done -> split across both engines.
                    for c in range(NSC):
                        if h == 1 and c % 2 == 1:
                            nc.scalar.copy(
                                HT[h][:, c * 512 : (c + 1) * 512], ph[c][:]
                            )
                        else:
                            nc.vector.tensor_copy(
                                HT[h][:, c * 512 : (c + 1) * 512], ph[c][:]
                            )

                def emit_pass2(h):
                    ph = [
                        pps.tile([128, 512], F32, name=f"ph{c}", tag=f"ph{c}", bufs=1)
                        for c in range(NSC)
                    ]
                    for i in range(NT):
                        emit_pass2_strip(h, ph, i)
                    emit_ht_copies(h, ph)

                _mark(nc, "PV+S0")
                # Interleave: strips' scores+exp first (feed ACT), V matmul
                # group fills PE during the exps, then the vsc muls that need
                # this group's V tiles. The last groups also hoist h1's first
                # exp strips into this phase's ACT slack so the S2h0+S1h1
                # phase is PE-bound instead of gating S2-h1 on the exp tail.
                HOIST = 2
                rects = {}
                for g in range(4):
                    for i in range(4 * g, 4 * g + 4):
                        emit_pass1_partA(0, i)
                    if g >= 2:
                        emit_pass1_partA(1, g - 2)
                    emit_v_group(g)
                    for i in range(4 * g, 4 * g + 4):
                        emit_pass1_partB(0, i)
                    if g >= 2:
                        emit_pass1_partB(1, g - 2)
                # a few pass2-h0 strips run at the tail of PV+S0 (PE-cheap
                # there) so the S2h0+S1h1 phase has less PE work than ACT
                # work -- otherwise the h1 exp stream finishes after pass2-h0
                # and S2-h1's first weight loads stall on it
                EARLY = 3
                ph0 = [
                    pps.tile([128, 512], F32, name=f"ph{c}", tag=f"ph{c}", bufs=1)
                    for c in range(NSC)
                ]
                for i in range(EARLY):
                    emit_pass2_strip(0, ph0, i)
                _mark(nc, "S2h0+S1h1")
                # pass2 of head 0 (PE-dense) strip-interleaved with pass1 of
                # head 1 (ACT-bound)
                for i in range(NT):
                    if i >= EARLY:
                        emit_pass2_strip(0, ph0, i)
                    if i >= HOIST:
                        emit_pass1_partA(1, i)
                        emit_pass1_partB(1, i)
                emit_ht_copies(0, ph0)
                _mark(nc, "S2-h1")
                emit_pass2(1)

            # ---------------- phase O: partial output projection --------------
            with tc.tile_pool(name="ppo", bufs=4, space="PSUM") as ppo:
                _mark(nc, "O")
                for st in range(NT):
                    # h-outer so each head's stationary HT strip is loaded
                    # once per st (the Ldweights-dedup BIR pass elides the
                    # repeats) instead of alternating HT0/HT1 every matmul.
                    pos = [
                        ppo.tile([128, 1024], F32, name="po", tag="po", bufs=4)
                        for _ in range(2)
                    ]
                    for h in range(HPC):
                        for cp in range(2):
                            for cc in range(2):
                                c = cp * 2 + cc
                                nc.tensor.matmul(
                                    pos[cp][:, cc * 512 : (cc + 1) * 512],
                                    HT[h][:, st * 128 : (st + 1) * 128],
                                    wo_sb[:, h * D + c * 512 : h * D + (c + 1) * 512],
                                    start=(h == 0),
                                    stop=(h == HPC - 1),
                                )
                    # Full-width [128, 2048] out tiles: 4KB DMA rows move the
                    # out stream from ~2KB descriptors to 4KB ones, roughly
                    # doubling effective write bandwidth so the final drain
                    # doesn't trail the last matmul by ~13us.
                    last = st == NT - 1
                    ot = opool.tile([128, 2048], BF16, name="ot", tag="ot", bufs=4)
                    nc.vector.tensor_copy(ot[:, 0:1024], pos[0][:])
                    nc.scalar.copy(ot[:, 1024:2048], pos[1][:])
                    if last:
                        # final tile: drain the two halves on BOTH DGE rings
                        # in parallel to halve the tail transfer
                        nc.scalar.dma_start(
                            out[st * 128 : (st + 1) * 128, 0:1024], ot[:, 0:1024]
                        )
                        nc.sync.dma_start(
                            out[st * 128 : (st + 1) * 128, 1024:2048], ot[:, 1024:2048]
                        )
                    else:
                        (nc.scalar if st % 2 == 0 else nc.sync).dma_start(
                            out[st * 128 : (st + 1) * 128, :], ot[:]
                        )

            if loop_ctx is not None:
                loop_ctx.__exit__(None, None, None)

    return nc


_NC = None


def _get_nc():
    global _NC
    if _NC is None:
        _NC = _build_bass()
    return _NC


def _prep_inputs(query, key, value, Wq, bq, Wk, bk, Wv, bv, Wo, bo):
    """Host-side shard + pack. Returns per-core input maps."""
    bf = ml_dtypes.bfloat16
    f32 = np.float32

    query = np.asarray(query, f32)
    key = np.asarray(key, f32)
    value = np.asarray(value, f32)
    Wq = np.asarray(Wq, f32)
    Wk = np.asarray(Wk, f32)
    Wv = np.asarray(Wv, f32)
    Wo = np.asarray(Wo, f32)
    bq = np.asarray(bq, f32)
    bk = np.asarray(bk, f32)
    bv = np.asarray(bv, f32)

    qT = np.ascontiguousarray(query.T).astype(bf)
    kT = np.ascontiguousarray(key.T).astype(bf)
    vT = np.ascontiguousarray(value.T).astype(bf)

    in_maps = []
    for c in range(N_CORES):
        heads = [c * HPC + j for j in range(HPC)]
        # wq/wk: [128, NT*HPC*128] strip-major: col = (dt*HPC + h)*128 + k,
        # row = d within tile (so a contiguous quarter = 4 strips, both heads)
        def pack_w(W):
            return np.concatenate(
                [
                    np.concatenate(
                        [W[hh].reshape(NT, 128, DK)[dt] for hh in heads], axis=1
                    )
                    for dt in range(NT)
                ],
                axis=1,
            ).astype(bf)

        # wv2: [128, NT*HPC*128], col = dt*(HPC*128) + h*128 + k
        wv2 = np.concatenate(
            [
                np.concatenate([Wv[hh].reshape(NT, 128, DK)[dt] for hh in heads], axis=1)
                for dt in range(NT)
            ],
            axis=1,
        ).astype(bf)

        wo_p = np.concatenate(
            [Wo[hh * DK : (hh + 1) * DK, :] for hh in heads], axis=1
        ).astype(bf)

        bqk = np.stack(
            [bq[hh] for hh in heads] + [bk[hh] for hh in heads], axis=1
        ).astype(f32)
        bvb = np.concatenate(
            [np.broadcast_to(bv[hh][None, :], (128, DK)) for hh in heads], axis=1
        ).astype(f32)

        in_maps.append(
            {
                "qT": qT,
                "kT": kT,
                "vT": vT,
                "wq": pack_w(Wq),
                "wk": pack_w(Wk),
                "wv2": np.ascontiguousarray(wv2),
                "wo": np.ascontiguousarray(wo_p),
                "bqk": np.ascontiguousarray(bqk),
                "bvb": np.ascontiguousarray(bvb),
            }
        )
    return in_maps


def kernel(query, key, value, Wq, bq, Wk, bk, Wv, bv, Wo, bo):
    global LAST_RESULTS
    in_maps = _prep_inputs(query, key, value, Wq, bq, Wk, bk, Wv, bv, Wo, bo)
    nc = _get_nc()
    res = bass_utils.run_bass_kernel_spmd(
        nc, in_maps, core_ids=list(range(N_CORES)), trace=TRACE
    )
    LAST_RESULTS = res
    acc = res.results[0]["out_p"].astype(np.float32)
    for c in range(1, N_CORES):
        acc += res.results[c]["out_p"].astype(np.float32)
    acc += np.asarray(bo, np.float32)[None, :]
    return acc

